# revision 43
# baseline (speedup 1.0000x reference)
"""Trainium2 Bass kernel for per-edge dot products (GNN DotPredictor).

out[e] = sum(h[src[e]] * h[dst[e]]); 800k edges, h [50k, 64] f32, 8 cores.

Design (v3):
  - Edges sharded 8 ways; h replicated. Per-edge rows fetched from HBM with
    the Q7 `dma_gather` path. The bottleneck is Pool-engine (Q7) descriptor
    generation (~9ns/descriptor per core pair, serial per pair), attacked on
    two axes:
    1. Descriptor count: edges sorted by (range-group, src); equal-src runs
       decomposed into K-edge units (K in {16,8,4,2,1}); one src descriptor
       of K*256B serves K edges (elem_step=64 overlapping rows). K=16 kept
       only in full pad-group multiples (rest demoted to K=8 pairs) so group
       padding stays cheap. dst side is one 256B descriptor per edge.
    2. Parallel generation: Bacc(num_swdge_queues=4) + queue_num=i%4 runs
       descriptor generation on all four Q7 core pairs concurrently
       (dma_gather ucode selects pair cpu_id/2 == queue_num); gathers are
       sliced into ~1024-descriptor pieces assigned round-robin so every
       window of 4 instructions covers all pairs. Measured ~3.2x over one
       queue (1090us -> ~343us); ~72% pair utilization is the practical cap
       (per-instruction pop/decode overhead on all 8 cores).
  - int16 gather indices => 4-way range bucketing (src>=32768, dst>=32768)
    with per-range base pointers; host permutes edges, unpermutes results.
  - A 128-idx warmup gather hides the ~6us Q7 IRAM library load under the
    initial index DMAs.
  - DVE: hu broadcast across K members via step-0 AP, in-place multiply
    into the hv tile, segment-reduce 64-feature dim to one score per edge.
  - Output [128, tiles] stored contiguously; host transposes + scatters.
"""

import os
from contextlib import ExitStack

import numpy as np

import concourse.bacc as bacc
import concourse.mybir as mybir
from concourse import library_config
from concourse.bass import AP
from concourse._compat import get_trn_type
from concourse.bass_utils import run_bass_kernel_spmd

N_NODES = 50000
NPAD = 50016  # h padded so K-row reads past the last node stay in bounds
D = 64
P = 128
N_CORES = 8
SPLIT = 32768

G_MAP = {16: 256, 8: 512, 4: 1024, 2: 2048, 1: 4096}  # units/chunk (<=4096 edges)

NBUF = 8  # hu/hv double-buffer depth
NQ = 4  # SWDGE queues (Q7 core pairs generating descriptors in parallel)

TRACE = False
LAST_RESULT = None



def _ensure_ntff_hook():
    """bass_utils' trace path imports antenv.axon_hooks, which this image's
    antenv package lacks. Recreate it from the boot helper so trace=True
    works; harmless no-op if the real module exists."""
    import sys
    import types

    try:
        import antenv.axon_hooks  # noqa: F401

        return
    except ImportError:
        pass
    try:
        import antenv
        from trn_agent_boot.trn_boot import _ntff_profile_via_ctypes

        hook = _ntff_profile_via_ctypes("/opt/axon/libaxon_pjrt.so")
        m = types.ModuleType("antenv.axon_hooks")
        m.get_axon_ntff_profile_hook = lambda: hook
        m.set_axon_ntff_profile_hook = lambda h: None
        sys.modules["antenv.axon_hooks"] = m
        antenv.axon_hooks = m
    except Exception:
        pass


def _wrap_idx(vals):
    """int16 index array [Npc] -> the [128, Npc/16] SBUF layout dma_gather
    expects (idx i at partition i%16, column i//16, replicated over the 8
    groups of 16 partitions)."""
    w = vals.reshape(-1, 16).T  # [16, Npc/16]
    return np.ascontiguousarray(np.tile(w, (8, 1)))  # [128, Npc/16]


def _host_prep(src, dst):
    """Sort by (range-group, src); decompose equal-src runs into K-units.

    Returns (schedule, seqs, sidx_per_core, didx_per_core, u_total, e_total):
      schedule: list of (K, s_hi, d_hi, u_off, e_off, n_units), same all cores
      seqs: [N_CORES, e_total] global edge id per output position (-1 pad)
    """
    E = src.shape[0]
    g = (src >= SPLIT).astype(np.int8) * 2 + (dst >= SPLIT).astype(np.int8)
    order0 = np.lexsort((src, g))
    sg, ss, sd = g[order0], src[order0], dst[order0]

    new = np.ones(E, bool)
    new[1:] = (sg[1:] != sg[:-1]) | (ss[1:] != ss[:-1])
    run_start = np.flatnonzero(new)
    d = np.diff(np.append(run_start, E))
    run_id = np.cumsum(new) - 1
    r = np.arange(E) - run_start[run_id]
    dd = d[run_id]
    n16 = (dd // 16) * 16
    n8 = n16 + (((dd - n16) // 8) * 8)
    n4 = n8 + (((dd - n8) // 4) * 4)
    n2 = n4 + (((dd - n4) // 2) * 2)
    K_e = np.where(
        r < n16,
        16,
        np.where(r < n8, 8, np.where(r < n4, 4, np.where(r < n2, 2, 1))),
    )
    m_e = np.where(
        K_e == 16, r % 16,
        np.where(
            K_e == 8, (r - n16) % 8,
            np.where(K_e == 4, (r - n8) % 4, np.where(K_e == 2, (r - n4) % 2, 0)),
        ),
    )
    first = m_e == 0

    pad_units = N_CORES * P
    # K=16 padding wastes 16 edges per pad unit; keep only full pad-group
    # multiples as K=16 and demote the rest to pairs of K=8 units.
    starts_map = {}
    for K in (16, 8, 4, 2, 1):
        for gg in range(4):
            starts_map[(K, gg)] = np.flatnonzero(
                first & (K_e == K) & (sg == gg)
            )
    for gg in range(4):
        s16 = starts_map[(16, gg)]
        keep = (s16.size // pad_units) * pad_units
        demoted = s16[keep:]
        starts_map[(16, gg)] = s16[:keep]
        if demoted.size:
            starts_map[(8, gg)] = np.sort(
                np.concatenate([starts_map[(8, gg)], demoted, demoted + 8])
            )

    schedule = []
    sidx_parts = [[] for _ in range(N_CORES)]
    didx_parts = [[] for _ in range(N_CORES)]
    seq_parts = [[] for _ in range(N_CORES)]
    u_off = 0
    e_off = 0
    for K in (16, 8, 4, 2, 1):
        for gg in range(4):
            starts = starts_map[(K, gg)]
            if starts.size == 0:
                continue
            Upad = -(-starts.size // pad_units) * pad_units
            buf = np.full(Upad, -1, dtype=np.int64)
            buf[: starts.size] = starts
            U = Upad // N_CORES  # per-core units, multiple of 128
            s_hi, d_hi = gg >= 2, gg % 2 == 1
            for c in range(N_CORES):
                uc = buf[c * U : (c + 1) * U]
                valid = uc >= 0
                sv = np.zeros(U, np.int64)
                sv[valid] = ss[uc[valid]] - (SPLIT if s_hi else 0)
                sidx_parts[c].append(sv.astype(np.int16))
                dvals = np.zeros(U * K, np.int64)
                ids = np.full(U * K, -1, np.int64)
                uu = np.arange(U)
                for m in range(K):
                    pos = (K * (uu // P) + m) * P + uu % P
                    dvals[pos[valid]] = sd[uc[valid] + m] - (
                        SPLIT if d_hi else 0
                    )
                    ids[pos[valid]] = order0[uc[valid] + m]
                didx_parts[c].append(dvals.astype(np.int16))
                seq_parts[c].append(ids)
            # chunks
            o, rem = 0, U
            Gn = G_MAP[K]
            while rem > 0:
                n = min(Gn, rem)
                schedule.append((K, s_hi, d_hi, u_off + o, e_off + o * K, n))
                o += n
                rem -= n
            u_off += U
            e_off += U * K

    seqs = np.stack([np.concatenate(p) for p in seq_parts])
    sidx = [np.concatenate(p) for p in sidx_parts]
    didx = [np.concatenate(p) for p in didx_parts]
    return schedule, seqs, sidx, didx, u_off, e_off


def _build_nc(schedule, u_total, e_total):
    SCOLS = u_total // 16
    DCOLS = e_total // 16
    TILES = e_total // P

    nc = bacc.Bacc(
        get_trn_type() or "TRN2",
        debug=False,
        dynamic_dma_scratch_size=32768,
        num_swdge_queues=NQ,
    )
    h = nc.dram_tensor("h", [NPAD, D], mybir.dt.float32, kind="ExternalInput")
    sidx = nc.dram_tensor("sidx", [P, SCOLS], mybir.dt.int16, kind="ExternalInput")
    didx = nc.dram_tensor("didx", [P, DCOLS], mybir.dt.int16, kind="ExternalInput")
    out = nc.dram_tensor("out", [P, TILES], mybir.dt.float32, kind="ExternalOutput")

    h_ap = h[:]
    # per-edge dst bases (rows of 64)
    hd_lo = h[0:SPLIT, :]
    hd_hi = h[SPLIT:NPAD, :]
    nch = len(schedule)

    with ExitStack() as stack:
        ent = stack.enter_context
        hu = [ent(nc.sbuf_tensor(f"hu{i}", [P, 2048], mybir.dt.float32)) for i in range(NBUF)]
        hv = [ent(nc.sbuf_tensor(f"hv{i}", [P, 2048], mybir.dt.float32)) for i in range(NBUF)]
        sidx_sb = ent(nc.sbuf_tensor("sidx_sb", [P, SCOLS], mybir.dt.int16))
        didx_sb = ent(nc.sbuf_tensor("didx_sb", [P, DCOLS], mybir.dt.int16))
        outb = ent(nc.sbuf_tensor("outb", [P, TILES], mybir.dt.float32))
        io = ent(nc.semaphore("io"))
        io2 = ent(nc.semaphore("io2"))
        gsem = [ent(nc.semaphore(f"g{i}")) for i in range(NBUF)]
        vsem = [ent(nc.semaphore(f"v{i}")) for i in range(NBUF)]
        mr = ent(nc.semaphore("mr"))

        def hu_ap(b, t_u, K):
            base = hu[b][:]
            return AP(base.tensor, 0, [[2048, P], [D * K, t_u], [1, D * K]])

        def hu_part_ap(b, blk_off, t_u, K):
            base = hu[b][:]
            return AP(
                base.tensor, blk_off * D * K, [[2048, P], [D * K, t_u], [1, D * K]]
            )

        def hu_bcast(b, t_u, K):
            base = hu[b][:]
            return AP(base.tensor, 0, [[2048, P], [D * K, t_u], [0, K], [1, D]])

        def hv_ap(b, t_e):
            base = hv[b][:]
            return AP(base.tensor, 0, [[2048, P], [D, t_e], [1, D]])

        def hv_part_ap(b, blk_off, t_e):
            base = hv[b][:]
            return AP(base.tensor, blk_off * D, [[2048, P], [D, t_e], [1, D]])

        def hv_4d(b, t_u, K):
            base = hv[b][:]
            return AP(base.tensor, 0, [[2048, P], [D * K, t_u], [D, K], [1, D]])

        def hsrc_ap(s_hi, K):
            if s_hi:
                return AP(h_ap.tensor, SPLIT * D, [[D, 17232], [1, D * K]])
            return AP(h_ap.tensor, 0, [[D, SPLIT], [1, D * K]])

        # Slice every gather (src and dst) into ~1024-descriptor pieces and
        # assign SWDGE queues round-robin by global piece index: each
        # consecutive window of NQ instructions then covers all NQ Q7 pairs
        # with near-equal work, which is what the Pool NX's shallow broadcast
        # FIFO needs to keep all pairs busy.
        PIECE = 1024
        pieces_per_chunk = []  # per chunk: list of (is_src, off, sz)
        for K, s_hi, d_hi, uo, eo, n in schedule:
            pieces = []
            for off in range(0, n, PIECE):
                pieces.append((True, off, min(PIECE, n - off)))
            ne = n * K
            for off in range(0, ne, PIECE):
                pieces.append((False, off, min(PIECE, ne - off)))
            pieces_per_chunk.append(pieces)

        # Emission order: src pieces of the first NBUF chunks go first (they
        # depend only on the small sidx DMA), so all four Q7 pairs have work
        # while the larger didx DMA still streams; the rest follows in chunk
        # order. Queues are assigned by emission index — strict rotation, so
        # consecutive pieces always hit distinct pairs (a least-loaded greedy
        # balanced totals better but raced intermittently on HW).
        emit_list = []  # [chunk, is_src, off, sz, queue]
        head = min(NBUF, nch)
        for c in range(head):
            for is_src, off, sz in pieces_per_chunk[c]:
                if is_src:
                    emit_list.append([c, True, off, sz])
        for c in range(head):
            for is_src, off, sz in pieces_per_chunk[c]:
                if not is_src:
                    emit_list.append([c, False, off, sz])
        for c in range(head, nch):
            for is_src, off, sz in pieces_per_chunk[c]:
                emit_list.append([c, is_src, off, sz])
        for i, e in enumerate(emit_list):
            e.append(i % NQ)

        # cumulative gsem value on buffer b after chunk c's gathers land
        # (gsem[0] starts at 16 from the warmup gather)
        gsem_target = []
        running = [16] + [0] * (NBUF - 1)
        for c, pieces in enumerate(pieces_per_chunk):
            b = c % NBUF
            running[b] += 16 * len(pieces)
            gsem_target.append(running[b])

        with nc.Block() as block:

            # first-half/rest column split of outb so the output DMA overlaps
            # the tail of the pipeline
            c_half = nch // 2
            half_col = schedule[c_half][4] // P  # eo of first chunk in 2nd half

            @block.sync
            def _(sync):
                sync.dma_start(sidx_sb[:], sidx[:]).then_inc(io, 16)
                sync.dma_start(didx_sb[:], didx[:]).then_inc(io, 16)
                for i in range(NBUF):
                    uses = sum(1 for c in range(c_half) if c % NBUF == i)
                    if uses > 0:
                        sync.wait_ge(vsem[i], uses)
                sync.dma_start(out[:, 0:half_col], outb[:, 0:half_col]).then_inc(
                    io2, 16
                )
                for i in range(NBUF):
                    uses = (nch - i + NBUF - 1) // NBUF
                    if uses > 0:
                        sync.wait_ge(vsem[i], uses)
                sync.dma_start(
                    out[:, half_col:TILES], outb[:, half_col:TILES]
                ).then_inc(io2, 16)
                sync.wait_ge(io2, 32)

            @block.gpsimd
            def _(gp):
                gp.load_library(library_config.mlp)
                # warm the dma_gather IRAM load while the (larger) didx DMA
                # still streams: a 128-idx gather on real sidx values
                gp.wait_ge(io, 16)
                gp.dma_gather(
                    hu_ap(0, 1, 1),
                    hsrc_ap(False, 1),
                    sidx_sb[:, 0:8],
                    P,
                    P,
                    D,
                    single_packet=False,
                ).then_inc(gsem[0], 16)
                didx_wait_done = False
                vsem_waited = set()
                for c, is_src, off, sz, q in emit_list:
                    K, s_hi, d_hi, uo, eo, n = schedule[c]
                    b = c % NBUF
                    if c >= NBUF and c not in vsem_waited:
                        gp.wait_ge(vsem[b], c // NBUF)
                        vsem_waited.add(c)
                    if not is_src and not didx_wait_done:
                        gp.wait_ge(io, 32)
                        didx_wait_done = True
                    if is_src:
                        gp.dma_gather(
                            hu_part_ap(b, off // P, sz // P, K),
                            hsrc_ap(s_hi, K),
                            sidx_sb[:, (uo + off) // 16 : (uo + off + sz) // 16],
                            sz,
                            sz,
                            D * K,
                            elem_step=D,
                            single_packet=False,
                            queue_num=q,
                        ).then_inc(gsem[b], 16)
                    else:
                        gp.dma_gather(
                            hv_part_ap(b, off // P, sz // P),
                            hd_hi if d_hi else hd_lo,
                            didx_sb[:, (eo + off) // 16 : (eo + off + sz) // 16],
                            sz,
                            sz,
                            D,
                            single_packet=False,
                            queue_num=q,
                        ).then_inc(gsem[b], 16)

            @block.vector
            def _(ve):
                for c, (K, s_hi, d_hi, uo, eo, n) in enumerate(schedule):
                    b = c % NBUF
                    ve.wait_ge(gsem[b], gsem_target[c])
                    t_u = n // P
                    t_e = t_u * K
                    if K == 1:
                        prod_in1 = hu_ap(b, t_u, 1)
                        prod = hv_ap(b, t_e)
                    else:
                        prod_in1 = hu_bcast(b, t_u, K)
                        prod = hv_4d(b, t_u, K)
                    ve.tensor_tensor(
                        out=prod, in0=prod, in1=prod_in1,
                        op=mybir.AluOpType.mult,
                    ).then_inc(mr, 1)
                    ve.wait_ge(mr, c + 1)
                    ve.tensor_reduce(
                        out=outb[:, eo // P : eo // P + t_e],
                        in_=prod,
                        axis=mybir.AxisListType.X,
                        op=mybir.AluOpType.add,
                    ).then_inc(vsem[b], 1)

    nc.compile()
    return nc


def kernel(h, src, dst):
    global LAST_RESULT
    h = np.asarray(h, dtype=np.float32)
    hp = np.zeros((NPAD, D), np.float32)
    hp[:N_NODES] = h
    src = np.asarray(src).astype(np.int64)
    dst = np.asarray(dst).astype(np.int64)
    E = src.shape[0]

    schedule, seqs, sidx, didx, u_total, e_total = _host_prep(src, dst)
    in_maps = [
        {"h": hp, "sidx": _wrap_idx(sidx[c]), "didx": _wrap_idx(didx[c])}
        for c in range(N_CORES)
    ]
    nc = _build_nc(schedule, u_total, e_total)

    if TRACE or os.environ.get("BASS_TRACE"):
        _ensure_ntff_hook()
    res = run_bass_kernel_spmd(nc, in_maps, core_ids=list(range(N_CORES)), trace=TRACE)
    LAST_RESULT = res

    out = np.empty(E, np.float32)
    for c in range(N_CORES):
        dots = res.results[c]["out"].T.reshape(-1)
        seq = seqs[c]
        valid = seq >= 0
        out[seq[valid]] = dots[valid]
    return out



# revision 44
# speedup vs baseline: 1.0127x; 1.0127x over previous
"""Trainium2 Bass kernel for per-edge dot products (GNN DotPredictor).

out[e] = sum(h[src[e]] * h[dst[e]]); 800k edges, h [50k, 64] f32, 8 cores.

Design (v3):
  - Edges sharded 8 ways; h replicated. Per-edge rows fetched from HBM with
    the Q7 `dma_gather` path. The bottleneck is Pool-engine (Q7) descriptor
    generation (~9ns/descriptor per core pair, serial per pair), attacked on
    two axes:
    1. Descriptor count: edges sorted by (range-group, src); equal-src runs
       decomposed into K-edge units (K in {16,8,4,2,1}); one src descriptor
       of K*256B serves K edges (elem_step=64 overlapping rows). K=16 kept
       only in full pad-group multiples (rest demoted to K=8 pairs) so group
       padding stays cheap. dst side is one 256B descriptor per edge.
    2. Parallel generation: Bacc(num_swdge_queues=4) + queue_num=i%4 runs
       descriptor generation on all four Q7 core pairs concurrently
       (dma_gather ucode selects pair cpu_id/2 == queue_num); gathers are
       sliced into ~1024-descriptor pieces assigned round-robin so every
       window of 4 instructions covers all pairs. Measured ~3.2x over one
       queue (1090us -> ~343us); ~72% pair utilization is the practical cap
       (per-instruction pop/decode overhead on all 8 cores).
  - int16 gather indices => 4-way range bucketing (src>=32768, dst>=32768)
    with per-range base pointers; host permutes edges, unpermutes results.
  - A 128-idx warmup gather hides the ~6us Q7 IRAM library load under the
    initial index DMAs.
  - DVE: hu broadcast across K members via step-0 AP, in-place multiply
    into the hv tile, segment-reduce 64-feature dim to one score per edge.
  - Output [128, tiles] stored contiguously; host transposes + scatters.
"""

import os
from contextlib import ExitStack

import numpy as np

import concourse.bacc as bacc
import concourse.mybir as mybir
from concourse import library_config
from concourse.bass import AP
from concourse._compat import get_trn_type
from concourse.bass_utils import run_bass_kernel_spmd

N_NODES = 50000
NPAD = 50016  # h padded so K-row reads past the last node stay in bounds
D = 64
P = 128
N_CORES = 8
SPLIT = 32768

G_MAP = {16: 256, 8: 512, 4: 1024, 2: 2048, 1: 4096}  # units/chunk (<=4096 edges)

NBUF = 8  # hu/hv double-buffer depth
NQ = 4  # SWDGE queues (Q7 core pairs generating descriptors in parallel)

TRACE = False
LAST_RESULT = None



def _ensure_ntff_hook():
    """bass_utils' trace path imports antenv.axon_hooks, which this image's
    antenv package lacks. Recreate it from the boot helper so trace=True
    works; harmless no-op if the real module exists."""
    import sys
    import types

    try:
        import antenv.axon_hooks  # noqa: F401

        return
    except ImportError:
        pass
    try:
        import antenv
        from trn_agent_boot.trn_boot import _ntff_profile_via_ctypes

        hook = _ntff_profile_via_ctypes("/opt/axon/libaxon_pjrt.so")
        m = types.ModuleType("antenv.axon_hooks")
        m.get_axon_ntff_profile_hook = lambda: hook
        m.set_axon_ntff_profile_hook = lambda h: None
        sys.modules["antenv.axon_hooks"] = m
        antenv.axon_hooks = m
    except Exception:
        pass


def _wrap_idx(vals):
    """int16 index array [Npc] -> the [128, Npc/16] SBUF layout dma_gather
    expects (idx i at partition i%16, column i//16, replicated over the 8
    groups of 16 partitions)."""
    w = vals.reshape(-1, 16).T  # [16, Npc/16]
    return np.ascontiguousarray(np.tile(w, (8, 1)))  # [128, Npc/16]


def _host_prep(src, dst):
    """Sort by (range-group, src); decompose equal-src runs into K-units.

    Returns (schedule, seqs, sidx_per_core, didx_per_core, u_total, e_total):
      schedule: list of (K, s_hi, d_hi, u_off, e_off, n_units), same all cores
      seqs: [N_CORES, e_total] global edge id per output position (-1 pad)
    """
    E = src.shape[0]
    g = (src >= SPLIT).astype(np.int8) * 2 + (dst >= SPLIT).astype(np.int8)
    order0 = np.lexsort((src, g))
    sg, ss, sd = g[order0], src[order0], dst[order0]

    new = np.ones(E, bool)
    new[1:] = (sg[1:] != sg[:-1]) | (ss[1:] != ss[:-1])
    run_start = np.flatnonzero(new)
    d = np.diff(np.append(run_start, E))
    run_id = np.cumsum(new) - 1
    r = np.arange(E) - run_start[run_id]
    dd = d[run_id]
    n16 = (dd // 16) * 16
    n8 = n16 + (((dd - n16) // 8) * 8)
    n4 = n8 + (((dd - n8) // 4) * 4)
    n2 = n4 + (((dd - n4) // 2) * 2)
    K_e = np.where(
        r < n16,
        16,
        np.where(r < n8, 8, np.where(r < n4, 4, np.where(r < n2, 2, 1))),
    )
    m_e = np.where(
        K_e == 16, r % 16,
        np.where(
            K_e == 8, (r - n16) % 8,
            np.where(K_e == 4, (r - n8) % 4, np.where(K_e == 2, (r - n4) % 2, 0)),
        ),
    )
    first = m_e == 0

    pad_units = N_CORES * P
    # K=16 padding wastes 16 edges per pad unit; keep only full pad-group
    # multiples as K=16 and demote the rest to pairs of K=8 units.
    starts_map = {}
    for K in (16, 8, 4, 2, 1):
        for gg in range(4):
            starts_map[(K, gg)] = np.flatnonzero(
                first & (K_e == K) & (sg == gg)
            )
    for gg in range(4):
        s16 = starts_map[(16, gg)]
        keep = (s16.size // pad_units) * pad_units
        demoted = s16[keep:]
        starts_map[(16, gg)] = s16[:keep]
        if demoted.size:
            starts_map[(8, gg)] = np.sort(
                np.concatenate([starts_map[(8, gg)], demoted, demoted + 8])
            )

    schedule = []
    sidx_parts = [[] for _ in range(N_CORES)]
    didx_parts = [[] for _ in range(N_CORES)]
    seq_parts = [[] for _ in range(N_CORES)]
    u_off = 0
    e_off = 0
    for K in (16, 8, 4, 2, 1):
        for gg in range(4):
            starts = starts_map[(K, gg)]
            if starts.size == 0:
                continue
            Upad = -(-starts.size // pad_units) * pad_units
            buf = np.full(Upad, -1, dtype=np.int64)
            buf[: starts.size] = starts
            U = Upad // N_CORES  # per-core units, multiple of 128
            s_hi, d_hi = gg >= 2, gg % 2 == 1
            for c in range(N_CORES):
                uc = buf[c * U : (c + 1) * U]
                valid = uc >= 0
                sv = np.zeros(U, np.int64)
                sv[valid] = ss[uc[valid]] - (SPLIT if s_hi else 0)
                sidx_parts[c].append(sv.astype(np.int16))
                dvals = np.zeros(U * K, np.int64)
                ids = np.full(U * K, -1, np.int64)
                uu = np.arange(U)
                for m in range(K):
                    pos = (K * (uu // P) + m) * P + uu % P
                    dvals[pos[valid]] = sd[uc[valid] + m] - (
                        SPLIT if d_hi else 0
                    )
                    ids[pos[valid]] = order0[uc[valid] + m]
                didx_parts[c].append(dvals.astype(np.int16))
                seq_parts[c].append(ids)
            # chunks
            o, rem = 0, U
            Gn = G_MAP[K]
            while rem > 0:
                n = min(Gn, rem)
                schedule.append((K, s_hi, d_hi, u_off + o, e_off + o * K, n))
                o += n
                rem -= n
            u_off += U
            e_off += U * K

    seqs = np.stack([np.concatenate(p) for p in seq_parts])
    sidx = [np.concatenate(p) for p in sidx_parts]
    didx = [np.concatenate(p) for p in didx_parts]
    return schedule, seqs, sidx, didx, u_off, e_off


def _build_nc(schedule, u_total, e_total):
    SCOLS = u_total // 16
    DCOLS = e_total // 16
    TILES = e_total // P

    nc = bacc.Bacc(
        get_trn_type() or "TRN2",
        debug=False,
        dynamic_dma_scratch_size=32768,
        num_swdge_queues=NQ,
    )
    h = nc.dram_tensor("h", [NPAD, D], mybir.dt.float32, kind="ExternalInput")
    sidx = nc.dram_tensor("sidx", [P, SCOLS], mybir.dt.int16, kind="ExternalInput")
    didx = nc.dram_tensor("didx", [P, DCOLS], mybir.dt.int16, kind="ExternalInput")
    out = nc.dram_tensor("out", [P, TILES], mybir.dt.float32, kind="ExternalOutput")

    h_ap = h[:]
    # per-edge dst bases (rows of 64)
    hd_lo = h[0:SPLIT, :]
    hd_hi = h[SPLIT:NPAD, :]
    nch = len(schedule)

    with ExitStack() as stack:
        ent = stack.enter_context
        hu = [ent(nc.sbuf_tensor(f"hu{i}", [P, 2048], mybir.dt.float32)) for i in range(NBUF)]
        hv = [ent(nc.sbuf_tensor(f"hv{i}", [P, 2048], mybir.dt.float32)) for i in range(NBUF)]
        sidx_sb = ent(nc.sbuf_tensor("sidx_sb", [P, SCOLS], mybir.dt.int16))
        didx_sb = ent(nc.sbuf_tensor("didx_sb", [P, DCOLS], mybir.dt.int16))
        outb = ent(nc.sbuf_tensor("outb", [P, TILES], mybir.dt.float32))
        io = ent(nc.semaphore("io"))
        io2 = ent(nc.semaphore("io2"))
        gsem = [ent(nc.semaphore(f"g{i}")) for i in range(NBUF)]
        vsem = [ent(nc.semaphore(f"v{i}")) for i in range(NBUF)]
        mr = ent(nc.semaphore("mr"))

        def hu_ap(b, t_u, K):
            base = hu[b][:]
            return AP(base.tensor, 0, [[2048, P], [D * K, t_u], [1, D * K]])

        def hu_part_ap(b, blk_off, t_u, K):
            base = hu[b][:]
            return AP(
                base.tensor, blk_off * D * K, [[2048, P], [D * K, t_u], [1, D * K]]
            )

        def hu_bcast(b, t_u, K):
            base = hu[b][:]
            return AP(base.tensor, 0, [[2048, P], [D * K, t_u], [0, K], [1, D]])

        def hv_ap(b, t_e):
            base = hv[b][:]
            return AP(base.tensor, 0, [[2048, P], [D, t_e], [1, D]])

        def hv_part_ap(b, blk_off, t_e):
            base = hv[b][:]
            return AP(base.tensor, blk_off * D, [[2048, P], [D, t_e], [1, D]])

        def hv_4d(b, t_u, K):
            base = hv[b][:]
            return AP(base.tensor, 0, [[2048, P], [D * K, t_u], [D, K], [1, D]])

        def hsrc_ap(s_hi, K):
            if s_hi:
                return AP(h_ap.tensor, SPLIT * D, [[D, 17232], [1, D * K]])
            return AP(h_ap.tensor, 0, [[D, SPLIT], [1, D * K]])

        # Slice every gather (src and dst) into ~1024-descriptor pieces and
        # assign SWDGE queues round-robin by global piece index: each
        # consecutive window of NQ instructions then covers all NQ Q7 pairs
        # with near-equal work, which is what the Pool NX's shallow broadcast
        # FIFO needs to keep all pairs busy.
        PIECE = 1024
        pieces_per_chunk = []  # per chunk: list of (is_src, off, sz, queue)
        piece_i = 0
        for K, s_hi, d_hi, uo, eo, n in schedule:
            pieces = []
            for off in range(0, n, PIECE):
                pieces.append([True, off, min(PIECE, n - off)])
            ne = n * K
            for off in range(0, ne, PIECE):
                pieces.append([False, off, min(PIECE, ne - off)])
            # strict rotation: consecutive pieces always hit distinct Q7
            # pairs (a least-loaded greedy balances totals better but showed
            # an intermittent data race on HW - do not reintroduce it)
            for p in pieces:
                p.append(piece_i % NQ)
                piece_i += 1
            pieces_per_chunk.append([tuple(p) for p in pieces])

        # cumulative gsem value on buffer b after chunk c's gathers land
        # (gsem[0] starts at 16 from the warmup gather)
        gsem_target = []
        running = [16] + [0] * (NBUF - 1)
        for c, pieces in enumerate(pieces_per_chunk):
            b = c % NBUF
            running[b] += 16 * len(pieces)
            gsem_target.append(running[b])

        with nc.Block() as block:

            # first-half/rest column split of outb so the output DMA overlaps
            # the tail of the pipeline
            c_half = nch // 2
            half_col = schedule[c_half][4] // P  # eo of first chunk in 2nd half

            @block.sync
            def _(sync):
                sync.dma_start(sidx_sb[:], sidx[:]).then_inc(io, 16)
                sync.dma_start(didx_sb[:], didx[:]).then_inc(io, 16)
                for i in range(NBUF):
                    uses = sum(1 for c in range(c_half) if c % NBUF == i)
                    if uses > 0:
                        sync.wait_ge(vsem[i], uses)
                sync.dma_start(out[:, 0:half_col], outb[:, 0:half_col]).then_inc(
                    io2, 16
                )
                for i in range(NBUF):
                    uses = (nch - i + NBUF - 1) // NBUF
                    if uses > 0:
                        sync.wait_ge(vsem[i], uses)
                sync.dma_start(
                    out[:, half_col:TILES], outb[:, half_col:TILES]
                ).then_inc(io2, 16)
                sync.wait_ge(io2, 32)

            @block.gpsimd
            def _(gp):
                gp.load_library(library_config.mlp)
                # warm the dma_gather IRAM load while the (larger) didx DMA
                # still streams: a 128-idx gather on real sidx values
                gp.wait_ge(io, 16)
                gp.dma_gather(
                    hu_ap(0, 1, 1),
                    hsrc_ap(False, 1),
                    sidx_sb[:, 0:8],
                    P,
                    P,
                    D,
                    single_packet=False,
                ).then_inc(gsem[0], 16)
                didx_wait_done = False
                for c, (K, s_hi, d_hi, uo, eo, n) in enumerate(schedule):
                    b = c % NBUF
                    if c >= NBUF:
                        gp.wait_ge(vsem[b], c // NBUF)
                    for is_src, off, sz, q in pieces_per_chunk[c]:
                        if not is_src and not didx_wait_done:
                            gp.wait_ge(io, 32)
                            didx_wait_done = True
                        if is_src:
                            gp.dma_gather(
                                hu_part_ap(b, off // P, sz // P, K),
                                hsrc_ap(s_hi, K),
                                sidx_sb[:, (uo + off) // 16 : (uo + off + sz) // 16],
                                sz,
                                sz,
                                D * K,
                                elem_step=D,
                                single_packet=False,
                                queue_num=q,
                            ).then_inc(gsem[b], 16)
                        else:
                            gp.dma_gather(
                                hv_part_ap(b, off // P, sz // P),
                                hd_hi if d_hi else hd_lo,
                                didx_sb[:, (eo + off) // 16 : (eo + off + sz) // 16],
                                sz,
                                sz,
                                D,
                                single_packet=False,
                                queue_num=q,
                            ).then_inc(gsem[b], 16)

            @block.vector
            def _(ve):
                for c, (K, s_hi, d_hi, uo, eo, n) in enumerate(schedule):
                    b = c % NBUF
                    ve.wait_ge(gsem[b], gsem_target[c])
                    t_u = n // P
                    t_e = t_u * K
                    if K == 1:
                        prod_in1 = hu_ap(b, t_u, 1)
                        prod = hv_ap(b, t_e)
                    else:
                        prod_in1 = hu_bcast(b, t_u, K)
                        prod = hv_4d(b, t_u, K)
                    ve.tensor_tensor(
                        out=prod, in0=prod, in1=prod_in1,
                        op=mybir.AluOpType.mult,
                    ).then_inc(mr, 1)
                    ve.wait_ge(mr, c + 1)
                    ve.tensor_reduce(
                        out=outb[:, eo // P : eo // P + t_e],
                        in_=prod,
                        axis=mybir.AxisListType.X,
                        op=mybir.AluOpType.add,
                    ).then_inc(vsem[b], 1)

    nc.compile()
    return nc


def kernel(h, src, dst):
    global LAST_RESULT
    h = np.asarray(h, dtype=np.float32)
    hp = np.zeros((NPAD, D), np.float32)
    hp[:N_NODES] = h
    src = np.asarray(src).astype(np.int64)
    dst = np.asarray(dst).astype(np.int64)
    E = src.shape[0]

    schedule, seqs, sidx, didx, u_total, e_total = _host_prep(src, dst)
    in_maps = [
        {"h": hp, "sidx": _wrap_idx(sidx[c]), "didx": _wrap_idx(didx[c])}
        for c in range(N_CORES)
    ]
    nc = _build_nc(schedule, u_total, e_total)

    if TRACE or os.environ.get("BASS_TRACE"):
        _ensure_ntff_hook()
    res = run_bass_kernel_spmd(nc, in_maps, core_ids=list(range(N_CORES)), trace=TRACE)
    LAST_RESULT = res

    out = np.empty(E, np.float32)
    for c in range(N_CORES):
        dots = res.results[c]["out"].T.reshape(-1)
        seq = seqs[c]
        valid = seq >= 0
        out[seq[valid]] = dots[valid]
    return out



# revision 48
# speedup vs baseline: 1.0174x; 1.0046x over previous
"""Trainium2 Bass kernel for per-edge dot products (GNN DotPredictor).

out[e] = sum(h[src[e]] * h[dst[e]]); 800k edges, h [50k, 64] f32, 8 cores.

Design (v3):
  - Edges sharded 8 ways; h replicated. Per-edge rows fetched from HBM with
    the Q7 `dma_gather` path. The bottleneck is Pool-engine (Q7) descriptor
    generation (~9ns/descriptor per core pair, serial per pair), attacked on
    two axes:
    1. Descriptor count: edges sorted by (range-group, src); equal-src runs
       decomposed into K-edge units (K in {16,8,4,2,1}); one src descriptor
       of K*256B serves K edges (elem_step=64 overlapping rows). K=16 kept
       only in full pad-group multiples (rest demoted to K=8 pairs) so group
       padding stays cheap. dst side is one 256B descriptor per edge.
    2. Parallel generation: Bacc(num_swdge_queues=4) + queue_num=i%4 runs
       descriptor generation on all four Q7 core pairs concurrently
       (dma_gather ucode selects pair cpu_id/2 == queue_num); gathers are
       sliced into ~1024-descriptor pieces assigned round-robin so every
       window of 4 instructions covers all pairs. Measured ~3.2x over one
       queue (1090us -> ~343us); ~72% pair utilization is the practical cap
       (per-instruction pop/decode overhead on all 8 cores).
  - int16 gather indices => 4-way range bucketing (src>=32768, dst>=32768)
    with per-range base pointers; host permutes edges, unpermutes results.
  - A 128-idx warmup gather hides the ~6us Q7 IRAM library load under the
    initial index DMAs.
  - DVE: hu broadcast across K members via step-0 AP, in-place multiply
    into the hv tile, segment-reduce 64-feature dim to one score per edge.
  - Output [128, tiles] stored contiguously; host transposes + scatters.
"""

import os
from contextlib import ExitStack

import numpy as np

import concourse.bacc as bacc
import concourse.mybir as mybir
from concourse import library_config
from concourse.bass import AP
from concourse._compat import get_trn_type
from concourse.bass_utils import run_bass_kernel_spmd

N_NODES = 50000
NPAD = 50016  # h padded so K-row reads past the last node stay in bounds
D = 64
P = 128
N_CORES = 8
SPLIT = 32768

G_MAP = {16: 256, 8: 512, 4: 1024, 2: 2048, 1: 4096}  # units/chunk (<=4096 edges)

NBUF = 8  # hu/hv double-buffer depth
NQ = 4  # SWDGE queues (Q7 core pairs generating descriptors in parallel)

TRACE = False
LAST_RESULT = None



def _ensure_ntff_hook():
    """bass_utils' trace path imports antenv.axon_hooks, which this image's
    antenv package lacks. Recreate it from the boot helper so trace=True
    works; harmless no-op if the real module exists."""
    import sys
    import types

    try:
        import antenv.axon_hooks  # noqa: F401

        return
    except ImportError:
        pass
    try:
        import antenv
        from trn_agent_boot.trn_boot import _ntff_profile_via_ctypes

        hook = _ntff_profile_via_ctypes("/opt/axon/libaxon_pjrt.so")
        m = types.ModuleType("antenv.axon_hooks")
        m.get_axon_ntff_profile_hook = lambda: hook
        m.set_axon_ntff_profile_hook = lambda h: None
        sys.modules["antenv.axon_hooks"] = m
        antenv.axon_hooks = m
    except Exception:
        pass


def _wrap_idx(vals):
    """int16 index array [Npc] -> the [128, Npc/16] SBUF layout dma_gather
    expects (idx i at partition i%16, column i//16, replicated over the 8
    groups of 16 partitions)."""
    w = vals.reshape(-1, 16).T  # [16, Npc/16]
    return np.ascontiguousarray(np.tile(w, (8, 1)))  # [128, Npc/16]


def _host_prep(src, dst):
    """Sort by (range-group, src); decompose equal-src runs into K-units.

    Returns (schedule, seqs, sidx_per_core, didx_per_core, u_total, e_total):
      schedule: list of (K, s_hi, d_hi, u_off, e_off, n_units), same all cores
      seqs: [N_CORES, e_total] global edge id per output position (-1 pad)
    """
    E = src.shape[0]
    g = (src >= SPLIT).astype(np.int8) * 2 + (dst >= SPLIT).astype(np.int8)
    order0 = np.lexsort((src, g))
    sg, ss, sd = g[order0], src[order0], dst[order0]

    new = np.ones(E, bool)
    new[1:] = (sg[1:] != sg[:-1]) | (ss[1:] != ss[:-1])
    run_start = np.flatnonzero(new)
    d = np.diff(np.append(run_start, E))
    run_id = np.cumsum(new) - 1
    r = np.arange(E) - run_start[run_id]
    dd = d[run_id]
    n16 = (dd // 16) * 16
    n8 = n16 + (((dd - n16) // 8) * 8)
    n4 = n8 + (((dd - n8) // 4) * 4)
    n2 = n4 + (((dd - n4) // 2) * 2)
    K_e = np.where(
        r < n16,
        16,
        np.where(r < n8, 8, np.where(r < n4, 4, np.where(r < n2, 2, 1))),
    )
    m_e = np.where(
        K_e == 16, r % 16,
        np.where(
            K_e == 8, (r - n16) % 8,
            np.where(K_e == 4, (r - n8) % 4, np.where(K_e == 2, (r - n4) % 2, 0)),
        ),
    )
    first = m_e == 0

    pad_units = N_CORES * P
    # K=16 padding wastes 16 edges per pad unit; keep only full pad-group
    # multiples as K=16 and demote the rest to pairs of K=8 units.
    starts_map = {}
    for K in (16, 8, 4, 2, 1):
        for gg in range(4):
            starts_map[(K, gg)] = np.flatnonzero(
                first & (K_e == K) & (sg == gg)
            )
    for gg in range(4):
        s16 = starts_map[(16, gg)]
        keep = (s16.size // pad_units) * pad_units
        demoted = s16[keep:]
        starts_map[(16, gg)] = s16[:keep]
        if demoted.size:
            starts_map[(8, gg)] = np.sort(
                np.concatenate([starts_map[(8, gg)], demoted, demoted + 8])
            )

    schedule = []
    sidx_parts = [[] for _ in range(N_CORES)]
    didx_parts = [[] for _ in range(N_CORES)]
    seq_parts = [[] for _ in range(N_CORES)]
    u_off = 0
    e_off = 0
    for K in (16, 8, 4, 2, 1):
        for gg in range(4):
            starts = starts_map[(K, gg)]
            if starts.size == 0:
                continue
            Upad = -(-starts.size // pad_units) * pad_units
            buf = np.full(Upad, -1, dtype=np.int64)
            buf[: starts.size] = starts
            U = Upad // N_CORES  # per-core units, multiple of 128
            s_hi, d_hi = gg >= 2, gg % 2 == 1
            for c in range(N_CORES):
                uc = buf[c * U : (c + 1) * U]
                valid = uc >= 0
                sv = np.zeros(U, np.int64)
                sv[valid] = ss[uc[valid]] - (SPLIT if s_hi else 0)
                sidx_parts[c].append(sv.astype(np.int16))
                dvals = np.zeros(U * K, np.int64)
                ids = np.full(U * K, -1, np.int64)
                uu = np.arange(U)
                for m in range(K):
                    pos = (K * (uu // P) + m) * P + uu % P
                    dvals[pos[valid]] = sd[uc[valid] + m] - (
                        SPLIT if d_hi else 0
                    )
                    ids[pos[valid]] = order0[uc[valid] + m]
                didx_parts[c].append(dvals.astype(np.int16))
                seq_parts[c].append(ids)
            # chunks
            o, rem = 0, U
            Gn = G_MAP[K]
            while rem > 0:
                n = min(Gn, rem)
                schedule.append((K, s_hi, d_hi, u_off + o, e_off + o * K, n))
                o += n
                rem -= n
            u_off += U
            e_off += U * K

    seqs = np.stack([np.concatenate(p) for p in seq_parts])
    sidx = [np.concatenate(p) for p in sidx_parts]
    didx = [np.concatenate(p) for p in didx_parts]
    return schedule, seqs, sidx, didx, u_off, e_off


def _build_nc(schedule, u_total, e_total):
    SCOLS = u_total // 16
    DCOLS = e_total // 16
    TILES = e_total // P

    nc = bacc.Bacc(
        get_trn_type() or "TRN2",
        debug=False,
        dynamic_dma_scratch_size=32768,
        num_swdge_queues=NQ,
    )
    h = nc.dram_tensor("h", [NPAD, D], mybir.dt.float32, kind="ExternalInput")
    sidx = nc.dram_tensor("sidx", [P, SCOLS], mybir.dt.int16, kind="ExternalInput")
    didx = nc.dram_tensor("didx", [P, DCOLS], mybir.dt.int16, kind="ExternalInput")
    out = nc.dram_tensor("out", [P, TILES], mybir.dt.float32, kind="ExternalOutput")

    h_ap = h[:]
    # per-edge dst bases (rows of 64)
    hd_lo = h[0:SPLIT, :]
    hd_hi = h[SPLIT:NPAD, :]
    nch = len(schedule)

    with ExitStack() as stack:
        ent = stack.enter_context
        hu = [ent(nc.sbuf_tensor(f"hu{i}", [P, 2048], mybir.dt.float32)) for i in range(NBUF)]
        hv = [ent(nc.sbuf_tensor(f"hv{i}", [P, 2048], mybir.dt.float32)) for i in range(NBUF)]
        sidx_sb = ent(nc.sbuf_tensor("sidx_sb", [P, SCOLS], mybir.dt.int16))
        didx_sb = ent(nc.sbuf_tensor("didx_sb", [P, DCOLS], mybir.dt.int16))
        outb = ent(nc.sbuf_tensor("outb", [P, TILES], mybir.dt.float32))
        io = ent(nc.semaphore("io"))
        io2 = ent(nc.semaphore("io2"))
        gsem = [ent(nc.semaphore(f"g{i}")) for i in range(NBUF)]
        vsem = [ent(nc.semaphore(f"v{i}")) for i in range(NBUF)]
        mr = ent(nc.semaphore("mr"))

        def hu_ap(b, t_u, K):
            base = hu[b][:]
            return AP(base.tensor, 0, [[2048, P], [D * K, t_u], [1, D * K]])

        def hu_part_ap(b, blk_off, t_u, K):
            base = hu[b][:]
            return AP(
                base.tensor, blk_off * D * K, [[2048, P], [D * K, t_u], [1, D * K]]
            )

        def hu_bcast(b, t_u, K):
            base = hu[b][:]
            return AP(base.tensor, 0, [[2048, P], [D * K, t_u], [0, K], [1, D]])

        def hv_ap(b, t_e):
            base = hv[b][:]
            return AP(base.tensor, 0, [[2048, P], [D, t_e], [1, D]])

        def hv_part_ap(b, blk_off, t_e):
            base = hv[b][:]
            return AP(base.tensor, blk_off * D, [[2048, P], [D, t_e], [1, D]])

        def hv_4d(b, t_u, K):
            base = hv[b][:]
            return AP(base.tensor, 0, [[2048, P], [D * K, t_u], [D, K], [1, D]])

        def hsrc_ap(s_hi, K):
            if s_hi:
                return AP(h_ap.tensor, SPLIT * D, [[D, 17232], [1, D * K]])
            return AP(h_ap.tensor, 0, [[D, SPLIT], [1, D * K]])

        # Slice every gather (src and dst) into ~1024-descriptor pieces and
        # assign SWDGE queues round-robin by global piece index: each
        # consecutive window of NQ instructions then covers all NQ Q7 pairs
        # with near-equal work, which is what the Pool NX's shallow broadcast
        # FIFO needs to keep all pairs busy.
        PIECE = 1024
        pieces_per_chunk = []  # per chunk: list of (is_src, off, sz, queue)
        piece_i = 0
        for K, s_hi, d_hi, uo, eo, n in schedule:
            pieces = []
            for off in range(0, n, PIECE):
                pieces.append([True, off, min(PIECE, n - off)])
            ne = n * K
            for off in range(0, ne, PIECE):
                pieces.append([False, off, min(PIECE, ne - off)])
            # strict rotation: consecutive pieces always hit distinct Q7
            # pairs (a least-loaded greedy balances totals better but showed
            # an intermittent data race on HW - do not reintroduce it)
            for p in pieces:
                p.append(piece_i % NQ)
                piece_i += 1
            pieces_per_chunk.append([tuple(p) for p in pieces])

        # cumulative gsem value on buffer b after chunk c's gathers land
        # (gsem[0] starts at 16 from the warmup gather)
        gsem_target = []
        running = [16] + [0] * (NBUF - 1)
        for c, pieces in enumerate(pieces_per_chunk):
            b = c % NBUF
            running[b] += 16 * len(pieces)
            gsem_target.append(running[b])

        with nc.Block() as block:

            # first-half/rest column split of outb so the output DMA overlaps
            # the tail of the pipeline
            c_half = nch // 2
            half_col = schedule[c_half][4] // P  # eo of first chunk in 2nd half

            @block.sync
            def _(sync):
                sync.dma_start(sidx_sb[:], sidx[:]).then_inc(io, 16)
                sync.dma_start(didx_sb[:], didx[:]).then_inc(io, 16)
                for i in range(NBUF):
                    uses = sum(1 for c in range(c_half) if c % NBUF == i)
                    if uses > 0:
                        sync.wait_ge(vsem[i], uses)
                sync.dma_start(out[:, 0:half_col], outb[:, 0:half_col]).then_inc(
                    io2, 16
                )
                for i in range(NBUF):
                    uses = (nch - i + NBUF - 1) // NBUF
                    if uses > 0:
                        sync.wait_ge(vsem[i], uses)
                sync.dma_start(
                    out[:, half_col:TILES], outb[:, half_col:TILES]
                ).then_inc(io2, 16)
                sync.wait_ge(io2, 32)

            @block.gpsimd
            def _(gp):
                gp.load_library(library_config.mlp)
                # warm the dma_gather IRAM load while the (larger) didx DMA
                # still streams: a 128-idx gather on real sidx values
                gp.wait_ge(io, 16)
                gp.dma_gather(
                    hu_ap(0, 1, 1),
                    hsrc_ap(False, 1),
                    sidx_sb[:, 0:8],
                    P,
                    P,
                    D,
                    single_packet=False,
                ).then_inc(gsem[0], 16)
                didx_wait_done = False
                for c, (K, s_hi, d_hi, uo, eo, n) in enumerate(schedule):
                    b = c % NBUF
                    if c >= NBUF:
                        gp.wait_ge(vsem[b], c // NBUF)
                    for is_src, off, sz, q in pieces_per_chunk[c]:
                        if not is_src and not didx_wait_done:
                            gp.wait_ge(io, 32)
                            didx_wait_done = True
                        if is_src:
                            gp.dma_gather(
                                hu_part_ap(b, off // P, sz // P, K),
                                hsrc_ap(s_hi, K),
                                sidx_sb[:, (uo + off) // 16 : (uo + off + sz) // 16],
                                sz,
                                sz,
                                D * K,
                                elem_step=D,
                                single_packet=False,
                                queue_num=q,
                            ).then_inc(gsem[b], 16)
                        else:
                            gp.dma_gather(
                                hv_part_ap(b, off // P, sz // P),
                                hd_hi if d_hi else hd_lo,
                                didx_sb[:, (eo + off) // 16 : (eo + off + sz) // 16],
                                sz,
                                sz,
                                D,
                                single_packet=False,
                                queue_num=q,
                            ).then_inc(gsem[b], 16)

            @block.vector
            def _(ve):
                for c, (K, s_hi, d_hi, uo, eo, n) in enumerate(schedule):
                    b = c % NBUF
                    ve.wait_ge(gsem[b], gsem_target[c])
                    t_u = n // P
                    t_e = t_u * K
                    if K == 1:
                        prod_in1 = hu_ap(b, t_u, 1)
                        prod = hv_ap(b, t_e)
                    else:
                        prod_in1 = hu_bcast(b, t_u, K)
                        prod = hv_4d(b, t_u, K)
                    ve.tensor_tensor(
                        out=prod, in0=prod, in1=prod_in1,
                        op=mybir.AluOpType.mult,
                    ).then_inc(mr, 1)
                    ve.wait_ge(mr, c + 1)
                    ve.tensor_reduce(
                        out=outb[:, eo // P : eo // P + t_e],
                        in_=prod,
                        axis=mybir.AxisListType.X,
                        op=mybir.AluOpType.add,
                    ).then_inc(vsem[b], 1)

    nc.compile()
    return nc


def kernel(h, src, dst):
    global LAST_RESULT
    h = np.asarray(h, dtype=np.float32)
    hp = np.zeros((NPAD, D), np.float32)
    hp[:N_NODES] = h
    src = np.asarray(src).astype(np.int64)
    dst = np.asarray(dst).astype(np.int64)
    E = src.shape[0]

    schedule, seqs, sidx, didx, u_total, e_total = _host_prep(src, dst)
    in_maps = [
        {"h": hp, "sidx": _wrap_idx(sidx[c]), "didx": _wrap_idx(didx[c])}
        for c in range(N_CORES)
    ]
    nc = _build_nc(schedule, u_total, e_total)

    if TRACE or os.environ.get("BASS_TRACE"):
        _ensure_ntff_hook()
    res = run_bass_kernel_spmd(nc, in_maps, core_ids=list(range(N_CORES)), trace=TRACE)
    LAST_RESULT = res

    out = np.empty(E, np.float32)
    for c in range(N_CORES):
        dots = res.results[c]["out"].T.reshape(-1)
        seq = seqs[c]
        valid = seq >= 0
        out[seq[valid]] = dots[valid]
    return out

